# revision 1
# baseline (speedup 1.0000x reference)
"""Trainium2 Bass kernel for the 2-layer GRU + BN + maxpool + FC model.

Strategy: time-shard across the 8 cores. The GRU state is strongly
contractive (validated: warmup of 32 steps reconverges to ~1e-7 rel in
fp32), so the 2048-step sequence is split into 16 windows of 128 output
steps, each preceded by a 32-step warmup from h=0. Each core processes 2
windows as independent interleaved serial chains over the full batch of
64, which amortizes the per-instruction fixed costs that dominate a
serial recurrence.

Inside a core, layer 2 lags layer 1 by one chunk (C=8 steps): per step
the PSUM tile holds L1 gates for step t and L2 gates for step t-C, so
the sigmoid/tanh/update ops are fused across both layers. The input
projections (gx = W_ih @ x and the h1->L2 projection) are batched per
chunk on the tensor engine. BatchNorm1 is folded into the L2 input
projection weights; BatchNorm2 + tanh + FC run on the host on the tiny
[128, 64] pooled result (max commutes with the positive-scale BN).
"""

import os
from contextlib import ExitStack

import numpy as np
import ml_dtypes

import concourse.bass as bass
import concourse.bacc as bacc
import concourse.tile as tile
from concourse import mybir
from concourse.bass_utils import run_bass_kernel_spmd

# Model dims (hardcoded per spec)
B, T, D, H1, H2, O = 64, 2048, 64, 256, 128, 10
EPS = 1e-5

# Schedule
NCORES = 8
KW = 2                      # windows per core
NWIN = NCORES * KW          # 16
WINLEN = T // NWIN          # 128 output steps per window
WU = 32                     # warmup steps
TW = WINLEN + WU            # 160 serial steps per window
C = 8                       # steps per chunk
NCH = TW // C               # 20 data chunks
NPC = NCH + 1               # +1 drain chunk (L2 lags L1 by one chunk)

BIG = 60000.0               # mask / z-clamp magnitude

# Compute dtype for weights/activations ("fp32" or "bf16"); PSUM is fp32.
DT_NAME = os.environ.get("GRU_DT", "fp32")


def _dts(dt_name):
    if dt_name == "bf16":
        return mybir.dt.bfloat16, ml_dtypes.bfloat16
    return mybir.dt.float32, np.float32


# psum step-layout column offsets (columns of the [128, 576] gate tile)
#   r: [L1r0 0:64 | L1r1 64:128 | L2r 128:192]
#   z: [L1z0 192:256 | L1z1 256:320 | L2z 320:384]
#   n: [L1n0 384:448 | L1n1 448:512 | L2n 512:576]
SEC_L1 = [0, 64, 192, 256, 384, 448]    # m-tile 0..5 = r0,r1,z0,z1,n0,n1
SEC_L2 = [128, 320, 512]                # m-tile 0..2 = r2,z2,n2
GW = 576                                # gate row width per step


def build_bass(dt_name=DT_NAME, tw=TW, c=C, kw=KW, wu=WU):
    """Build the SPMD single-core program (same on all 8 cores)."""
    DT, _ = _dts(dt_name)
    F32 = mybir.dt.float32
    nch = tw // c
    npc = nch + 1

    nc = bacc.Bacc("TRN2", target_bir_lowering=False, debug=False,
                   num_devices=NCORES)

    # ---- DRAM I/O (per-core data; program identical across cores) ----
    xts = [nc.dram_tensor(f"xt{w}", [D + 1, (tw + c) * B], DT,
                          kind="ExternalInput").ap() for w in range(kw)]
    wa_d = nc.dram_tensor("wa", [D + 1, 3 * H1], DT, kind="ExternalInput").ap()
    whh1_d = nc.dram_tensor("whh1", [H1, 3 * H1], DT, kind="ExternalInput").ap()
    wb_d = nc.dram_tensor("wb", [H1, 3 * H2], DT, kind="ExternalInput").ap()
    b2row_d = nc.dram_tensor("b2row", [1, 3 * H2], DT, kind="ExternalInput").ap()
    whh2_d = nc.dram_tensor("whh2", [H2, 3 * H2], DT, kind="ExternalInput").ap()
    bhn1_d = nc.dram_tensor("bhn1", [1, H1], DT, kind="ExternalInput").ap()
    bhn2_d = nc.dram_tensor("bhn2", [1, H2], DT, kind="ExternalInput").ap()
    idn_d = nc.dram_tensor("idn", [128, 128], DT, kind="ExternalInput").ap()
    maskb_d = nc.dram_tensor("maskb", [128, kw * npc], F32,
                             kind="ExternalInput").ap()
    pmax_d = nc.dram_tensor("pmax", [128, B], F32, kind="ExternalOutput").ap()

    with tile.TileContext(nc) as tc, ExitStack() as ctx:
        singles = ctx.enter_context(tc.tile_pool(name="singles", bufs=1))
        work = ctx.enter_context(tc.tile_pool(name="work", bufs=2))
        xcp = ctx.enter_context(tc.tile_pool(name="xc", bufs=3))
        gxp = ctx.enter_context(tc.tile_pool(name="gx", bufs=2))
        hbp = ctx.enter_context(tc.tile_pool(name="hb", bufs=2))
        gpsA = ctx.enter_context(tc.tile_pool(name="gpsA", bufs=1, space="PSUM"))
        gpsB = ctx.enter_context(tc.tile_pool(name="gpsB", bufs=1, space="PSUM"))
        php = ctx.enter_context(tc.tile_pool(name="php", bufs=2, space="PSUM"))
        gates_pool = [gpsA, gpsB]

        # ---- load constants into SBUF ----
        wa_sb = singles.tile([D + 1, 3 * H1], DT)
        nc.sync.dma_start(wa_sb[:], wa_d[:])
        whh1_sb = singles.tile([128, 2 * 3 * H1], DT)  # [128, 1536]: k0|k1
        nc.sync.dma_start(whh1_sb[:, 0:768], whh1_d[0:128, :])
        nc.sync.dma_start(whh1_sb[:, 768:1536], whh1_d[128:256, :])
        wb_sb = singles.tile([128, 2 * 3 * H2], DT)    # [128, 768]: k0|k1
        nc.sync.dma_start(wb_sb[:, 0:384], wb_d[0:128, :])
        nc.sync.dma_start(wb_sb[:, 384:768], wb_d[128:256, :])
        b2row_sb = singles.tile([1, 3 * H2], DT)
        nc.sync.dma_start(b2row_sb[:], b2row_d[:])
        whh2_sb = singles.tile([H2, 3 * H2], DT)
        nc.sync.dma_start(whh2_sb[:], whh2_d[:])
        bhn1_sb = singles.tile([1, H1], DT)
        nc.sync.dma_start(bhn1_sb[:], bhn1_d[:])
        bhn2_sb = singles.tile([1, H2], DT)
        nc.sync.dma_start(bhn2_sb[:], bhn2_d[:])
        idn_sb = singles.tile([128, 128], DT)
        nc.sync.dma_start(idn_sb[:], idn_d[:])
        maskb_sb = singles.tile([128, kw * npc], F32)
        nc.sync.dma_start(maskb_sb[:], maskb_d[:])
        ones_sb = singles.tile([1, c * B], DT)
        nc.vector.memset(ones_sb[:], 1.0)
        pmax_sb = [singles.tile([128, B], F32, tag=f"pmax{w}", name=f"pmax{w}") for w in range(kw)]
        for w in range(kw):
            nc.vector.memset(pmax_sb[w][:], -2.0 * BIG)

        def phase_a(w, k, gxc, xc):
            """gx1 chunk k for window w: 6 matmuls + 6 copies into gxc L1 sections."""
            for m in range(6):
                ps = php.tile([128, c * B], F32, tag="ph", name="ph")
                nc.tensor.matmul(ps[:], wa_sb[:, m * 128:(m + 1) * 128], xc[:],
                                 start=True, stop=True)
                nc.vector.tensor_copy(
                    gxc[:, :, SEC_L1[m]:SEC_L1[m] + B],
                    ps[:].rearrange("p (t b) -> p t b", t=c))

        def phase_b(w, k, gxc, hb):
            """gx2 for L2 steps k*C..k*C+C-1 from h1 written in chunk k; writes
            gxc(k+1) L2 sections. 9 matmuls + 3 copies."""
            h1k0 = hb[:, 1:c + 1, 0:B]
            h1k1 = hb[:, 1:c + 1, B:2 * B]
            for m in range(3):
                ps = php.tile([128, c * B], F32, tag="ph", name="ph")
                nc.tensor.matmul(ps[:], wb_sb[:, m * 128:(m + 1) * 128], h1k0,
                                 start=True, stop=False)
                nc.tensor.matmul(ps[:], wb_sb[:, 384 + m * 128:384 + (m + 1) * 128],
                                 h1k1, start=False, stop=False)
                nc.tensor.matmul(ps[:], b2row_sb[:, m * 128:(m + 1) * 128],
                                 ones_sb[:], start=False, stop=True)
                nc.vector.tensor_copy(
                    gxc[:, :, SEC_L2[m]:SEC_L2[m] + B],
                    ps[:].rearrange("p (t b) -> p t b", t=c))

        # ---- prologue ----
        xcs = [[None] * (nch + 1) for _ in range(kw)]
        for w in range(kw):
            for k in (0, 1):
                xcs[w][k] = xcp.tile([D + 1, c * B], DT, tag=f"xc{w}", name=f"xc{w}")
                nc.sync.dma_start(xcs[w][k][:], xts[w][:, k * c * B:(k + 1) * c * B])
        gxc_cur = [None] * kw
        hb_cur = [None] * kw
        for w in range(kw):
            g = gxp.tile([128, c, GW], DT, tag=f"gx{w}", name=f"gx{w}")
            nc.vector.memset(g[:], 0.0)
            # junk chunk 0: clamp L2 z-gate input so z=1 and h2 stays exactly 0
            nc.vector.memset(g[:, :, SEC_L2[1]:SEC_L2[1] + B], BIG)
            phase_a(w, 0, g, xcs[w][0])
            gxc_cur[w] = g
            h = hbp.tile([128, c + 1, 192], DT, tag=f"hb{w}", name=f"hb{w}")
            nc.vector.memset(h[:, 0, :], 0.0)
            hb_cur[w] = h

        # ---- main loop over processing chunks ----
        for k in range(npc):
            gxc_next = [None] * kw
            # prefetch x for chunk k+2; phase-A for chunk k+1
            for w in range(kw):
                if k + 2 <= nch:
                    xcs[w][k + 2] = xcp.tile([D + 1, c * B], DT, tag=f"xc{w}", name=f"xc{w}")
                    nc.sync.dma_start(xcs[w][k + 2][:],
                                      xts[w][:, (k + 2) * c * B:(k + 3) * c * B])
                if k + 1 <= nch:
                    g = gxp.tile([128, c, GW], DT, tag=f"gx{w}", name=f"gx{w}")
                    phase_a(w, k + 1, g, xcs[w][k + 1])
                    gxc_next[w] = g

            # steps
            for s in range(c):
                Pr = [gates_pool[w].tile([128, 192], F32, tag=f"Pr{w}", name=f"Pr{w}")
                      for w in range(kw)]
                Pz = [gates_pool[w].tile([128, 192], F32, tag=f"Pz{w}", name=f"Pz{w}")
                      for w in range(kw)]
                Pn = [gates_pool[w].tile([128, 192], F32, tag=f"Pn{w}", name=f"Pn{w}")
                      for w in range(kw)]
                srz = [work.tile([128, 384], DT, tag=f"srz{w}", name=f"srz{w}") for w in range(kw)]
                tn = [work.tile([128, 192], DT, tag=f"tn{w}", name=f"tn{w}") for w in range(kw)]
                tn2 = [work.tile([128, 192], DT, tag=f"tn2{w}", name=f"tn2{w}") for w in range(kw)]
                ntl = [work.tile([128, 192], DT, tag=f"ntl{w}", name=f"ntl{w}") for w in range(kw)]
                wzh = [work.tile([128, 192], DT, tag=f"wzh{w}", name=f"wzh{w}") for w in range(kw)]
                u = [work.tile([128, 192], DT, tag=f"u{w}", name=f"u{w}") for w in range(kw)]

                def hslice(w, lo, hi):
                    return hb_cur[w][:, s, lo:hi]

                # ---- r bank: gx prefill + W_hr matmuls, close, sigmoid ----
                for w in range(kw):
                    nc.tensor.matmul(Pr[w][:], idn_sb[:], gxc_cur[w][:, s, 0:192],
                                     start=True, stop=False)
                for m in (0, 1):  # L1 r0, r1
                    for w in range(kw):
                        nc.tensor.matmul(Pr[w][:, m * B:(m + 1) * B],
                                         whh1_sb[:, m * 128:(m + 1) * 128],
                                         hslice(w, 0, B), start=False, stop=False)
                        nc.tensor.matmul(Pr[w][:, m * B:(m + 1) * B],
                                         whh1_sb[:, 768 + m * 128:768 + (m + 1) * 128],
                                         hslice(w, B, 2 * B), start=False, stop=False)
                for w in range(kw):  # L2 r (last write to r bank)
                    nc.tensor.matmul(Pr[w][:, 2 * B:3 * B],
                                     whh2_sb[:, 0:128], hslice(w, 2 * B, 3 * B),
                                     start=False, stop=True)
                for w in range(kw):
                    nc.scalar.activation(srz[w][:, 0:192], Pr[w][:],
                                         mybir.ActivationFunctionType.Sigmoid)
                # ---- n bank ----
                for m in (4, 5):  # L1 n0, n1
                    for w in range(kw):
                        nc.tensor.matmul(Pn[w][:, (m - 4) * B:(m - 3) * B],
                                         whh1_sb[:, m * 128:(m + 1) * 128],
                                         hslice(w, 0, B),
                                         start=(m == 4), stop=False)
                        nc.tensor.matmul(Pn[w][:, (m - 4) * B:(m - 3) * B],
                                         whh1_sb[:, 768 + m * 128:768 + (m + 1) * 128],
                                         hslice(w, B, 2 * B), start=False, stop=False)
                        nc.tensor.matmul(Pn[w][:, (m - 4) * B:(m - 3) * B],
                                         bhn1_sb[:, (m - 4) * 128:(m - 3) * 128],
                                         ones_sb[:, 0:B], start=False, stop=False)
                for w in range(kw):  # L2 n (last writes to n bank)
                    nc.tensor.matmul(Pn[w][:, 2 * B:3 * B],
                                     whh2_sb[:, 256:384], hslice(w, 2 * B, 3 * B),
                                     start=False, stop=False)
                    nc.tensor.matmul(Pn[w][:, 2 * B:3 * B],
                                     bhn2_sb[:], ones_sb[:, 0:B],
                                     start=False, stop=True)
                # ---- z bank ----
                for w in range(kw):
                    nc.tensor.matmul(Pz[w][:], idn_sb[:], gxc_cur[w][:, s, 192:384],
                                     start=True, stop=False)
                for m in (2, 3):  # L1 z0, z1
                    for w in range(kw):
                        nc.tensor.matmul(Pz[w][:, (m - 2) * B:(m - 1) * B],
                                         whh1_sb[:, m * 128:(m + 1) * 128],
                                         hslice(w, 0, B), start=False, stop=False)
                        nc.tensor.matmul(Pz[w][:, (m - 2) * B:(m - 1) * B],
                                         whh1_sb[:, 768 + m * 128:768 + (m + 1) * 128],
                                         hslice(w, B, 2 * B), start=False, stop=False)
                for w in range(kw):  # L2 z (last write to z bank)
                    nc.tensor.matmul(Pz[w][:, 2 * B:3 * B],
                                     whh2_sb[:, 128:256], hslice(w, 2 * B, 3 * B),
                                     start=False, stop=True)
                for w in range(kw):
                    nc.scalar.activation(srz[w][:, 192:384], Pz[w][:],
                                         mybir.ActivationFunctionType.Sigmoid)
                # n path + state update
                for w in range(kw):
                    nc.vector.tensor_mul(tn[w][:], srz[w][:, 0:192], Pn[w][:])
                for w in range(kw):
                    nc.vector.tensor_add(tn2[w][:], tn[w][:],
                                         gxc_cur[w][:, s, 384:576])
                for w in range(kw):
                    nc.scalar.activation(ntl[w][:], tn2[w][:],
                                         mybir.ActivationFunctionType.Tanh)
                for w in range(kw):
                    nc.gpsimd.tensor_mul(wzh[w][:], srz[w][:, 192:384],
                                         hb_cur[w][:, s, :])
                for w in range(kw):
                    nc.vector.scalar_tensor_tensor(
                        u[w][:], srz[w][:, 192:384], 1.0, ntl[w][:],
                        op0=mybir.AluOpType.subtract, op1=mybir.AluOpType.mult)
                for w in range(kw):
                    nc.vector.tensor_sub(hb_cur[w][:, s + 1, :], wzh[w][:], u[w][:])

            # phase-B, carry, pooling
            for w in range(kw):
                if k <= nch - 1:
                    phase_b(w, k, gxc_next[w], hb_cur[w])
            for w in range(kw):
                if k + 1 <= npc - 1:
                    hnew = hbp.tile([128, c + 1, 192], DT, tag=f"hb{w}", name=f"hb{w}")
                    nc.gpsimd.tensor_copy(hnew[:, 0, :], hb_cur[w][:, c, :])
                else:
                    hnew = None
                # pooling over this chunk's h2 (L2 steps (k-1)*C .. k*C-1)
                cmax = work.tile([128, B], F32, tag=f"cmax{w}", name=f"cmax{w}")
                nc.vector.tensor_reduce(
                    cmax[:], hb_cur[w][:, 1:c + 1, 128:192].rearrange("p t b -> p b t"),
                    axis=mybir.AxisListType.X, op=mybir.AluOpType.max)
                nc.vector.tensor_scalar(cmax[:], cmax[:],
                                        maskb_sb[:, w * npc + k:w * npc + k + 1],
                                        None, op0=mybir.AluOpType.add)
                nc.vector.tensor_max(pmax_sb[w][:], pmax_sb[w][:], cmax[:])
                if hnew is not None:
                    hb_cur[w] = hnew
                gxc_cur[w] = gxc_next[w]

        # ---- epilogue ----
        nc.vector.tensor_max(pmax_sb[0][:], pmax_sb[0][:], pmax_sb[1][:])
        nc.sync.dma_start(pmax_d[:], pmax_sb[0][:])

    nc.compile()
    return nc


def prep_core_inputs(inputs, dt_name=DT_NAME, tw=TW, c=C, kw=KW, wu=WU,
                     winlen=WINLEN):
    """Host-side data prep: per-core input dicts (layout/slice/cast only)."""
    _, NPD = _dts(dt_name)
    nch = tw // c
    npc = nch + 1
    x = np.asarray(inputs['x'], np.float32)
    W_ih1 = np.asarray(inputs['W_ih1'], np.float32)
    W_hh1 = np.asarray(inputs['W_hh1'], np.float32)
    b_ih1 = np.asarray(inputs['b_ih1'], np.float32)
    b_hh1 = np.asarray(inputs['b_hh1'], np.float32)
    W_ih2 = np.asarray(inputs['W_ih2'], np.float32)
    W_hh2 = np.asarray(inputs['W_hh2'], np.float32)
    b_ih2 = np.asarray(inputs['b_ih2'], np.float32)
    b_hh2 = np.asarray(inputs['b_hh2'], np.float32)
    g1, be1 = np.asarray(inputs['bn1_gamma'], np.float32), np.asarray(inputs['bn1_beta'], np.float32)
    m1, v1 = np.asarray(inputs['bn1_mean'], np.float32), np.asarray(inputs['bn1_var'], np.float32)

    s1 = g1 / np.sqrt(v1 + EPS)
    W2p = W_ih2 * s1[None, :]                      # [384, 256] scaled
    b2extra = W_ih2 @ (be1 - m1 * s1)              # [384]
    b2row = (b2extra + b_ih2 +
             np.concatenate([b_hh2[0:H2], b_hh2[H2:2 * H2], np.zeros(H2, np.float32)]))
    wa = np.vstack([
        W_ih1.T,
        (b_ih1 + np.concatenate([b_hh1[0:H1], b_hh1[H1:2 * H1],
                                 np.zeros(H1, np.float32)]))[None, :],
    ])  # [65, 768]

    base = dict(
        wa=wa.astype(NPD),
        whh1=W_hh1.T.astype(NPD).copy(),
        wb=W2p.T.astype(NPD).copy(),
        b2row=b2row[None, :].astype(NPD),
        whh2=W_hh2.T.astype(NPD).copy(),
        bhn1=b_hh1[2 * H1:3 * H1][None, :].astype(NPD),
        bhn2=b_hh2[2 * H2:3 * H2][None, :].astype(NPD),
        idn=np.eye(128, dtype=np.float32).astype(NPD),
    )

    n_win = (T // winlen)
    in_maps = []
    for core in range(NCORES):
        m = dict(base)
        maskb = np.zeros((128, kw * npc), np.float32)
        for w in range(kw):
            widx = core * kw + w
            t0 = 0 if widx == 0 else widx * winlen - wu
            xw = x[:, t0:t0 + tw, :]                       # [64, TW, 64]
            xt = np.transpose(xw, (2, 1, 0)).reshape(D, tw * B)
            xt = np.concatenate([xt, np.zeros((D, c * B), np.float32)], axis=1)
            xt = np.vstack([xt, np.ones((1, (tw + c) * B), np.float32)])
            m[f"xt{w}"] = np.ascontiguousarray(xt).astype(NPD)
            maskb[:, w * npc + 0] = -BIG                   # junk L2 chunk
            if widx > 0:
                maskb[:, w * npc + 1:w * npc + 1 + wu // c] = -BIG
        m["maskb"] = maskb
        in_maps.append(m)
    return in_maps


def finalize(pmax_list, inputs):
    """Host: combine per-core pooled maxima, apply BN2 + tanh + FC."""
    pmax = np.max(np.stack(pmax_list), axis=0)             # [128, 64]
    g2 = np.asarray(inputs['bn2_gamma'], np.float32)
    be2 = np.asarray(inputs['bn2_beta'], np.float32)
    m2 = np.asarray(inputs['bn2_mean'], np.float32)
    v2 = np.asarray(inputs['bn2_var'], np.float32)
    fc_w = np.asarray(inputs['fc_w'], np.float32)
    fc_b = np.asarray(inputs['fc_b'], np.float32)
    s2 = g2 / np.sqrt(v2 + EPS)
    th = np.tanh(pmax * s2[:, None] + (be2 - m2 * s2)[:, None])   # [128, 64]
    return (th.T @ fc_w.T + fc_b).astype(np.float32)               # [64, 10]


_NC_CACHE = {}


def _get_nc(dt_name=DT_NAME):
    if dt_name not in _NC_CACHE:
        _NC_CACHE[dt_name] = build_bass(dt_name)
    return _NC_CACHE[dt_name]


def kernel(**inputs):
    nc = _get_nc()
    in_maps = prep_core_inputs(inputs)
    res = run_bass_kernel_spmd(nc, in_maps, list(range(NCORES)))
    pmax_list = [res.results[i]["pmax"] for i in range(NCORES)]
    return finalize(pmax_list, inputs)



# revision 7
# speedup vs baseline: 1.5537x; 1.5537x over previous
"""Trainium2 Bass kernel for the 2-layer GRU + BN + maxpool + FC model.

Strategy: time-shard across the 8 cores. The GRU state is strongly
contractive (warmup of 16 steps reconverges to ~2.5e-4 rel in fp32), so
the 2048-step sequence is split into 16 windows of 128 output steps,
each preceded by a 16-step warmup from h=0. Each core processes 2
windows over the full batch of 64, BATCHED side by side into 128-col
matmul rhs tiles so every per-step tensor instruction covers both
windows at once.

Layouts (all per core):
  h state  hb[128, c+1, 6, 64]: sections = h1a_w0|h1a_w1|h1b_w0|h1b_w1|h2_w0|h2_w1
  gates    Pr/Pz/Pn PSUM [128, 384]: sections = m0_w0|m0_w1|m1_w0|m1_w1|L2_w0|L2_w1
  gx       gxc[128, c, 3, 384] (gate-major, same 384-col section layout)

Per step: gx is injected into the r/z PSUM banks with one wide identity
matmul each; the n-gate hidden bias enters through a K=3 selector
matmul; L1 uses 4 matmuls per bank (2 m-tiles x 2 k-halves) and L2 one.
Input projections (phase_a: W_ih1 @ x, phase_b: h1 -> L2 gx) are batched
per chunk; phase_b's bias folds into the PSUM->SBUF copies
(tensor_scalar add). BatchNorm1 is folded into the L2 input projection;
BatchNorm2 + tanh + FC run on the host on the tiny pooled result (max
commutes with the positive-scale BN).

Compute dtype bf16 (PSUM accumulation fp32): single-pass PE matmuls vs
fp32's LOW_HIGH dual pass; rel err ~7e-3 vs the 2e-2 gate.
"""

import os
from contextlib import ExitStack

import numpy as np
import ml_dtypes

import concourse.bass as bass
import concourse.bacc as bacc
import concourse.tile as tile
from concourse import mybir
from concourse.bass_utils import run_bass_kernel_spmd

# Model dims (hardcoded per spec)
B, T, D, H1, H2, O = 64, 2048, 64, 256, 128, 10
EPS = 1e-5

# Schedule
NCORES = 8
KW = 2                      # windows per core (batched into one chain)
NWIN = NCORES * KW          # 16
WINLEN = T // NWIN          # 128 output steps per window
WU = int(os.environ.get("GRU_WU", "16"))   # warmup steps
TW = WINLEN + WU            # serial steps per window
C = 8                       # steps per chunk
NCH = TW // C               # data chunks
NPC = NCH + 1               # +1 drain chunk (L2 lags L1 by one chunk)

BIG = 60000.0               # mask / z-clamp magnitude

# Compute dtype for weights/activations ("fp32" or "bf16"); PSUM is fp32.
DT_NAME = os.environ.get("GRU_DT", "bf16")
# Inject gx into r/z banks via "tensor" (identity matmul) or "vector" add
GX_VIA = os.environ.get("GRU_GX", "tensor")


def _dts(dt_name):
    if dt_name == "bf16":
        return mybir.dt.bfloat16, ml_dtypes.bfloat16
    return mybir.dt.float32, np.float32

SW = 3 * KW * B             # 384: one gate-bank width (2 L1 m-tiles + L2)
GW = 3 * SW                 # 1152: gx row width per step (3 gates x 384)


def build_bass(dt_name=DT_NAME, tw=TW, c=C, wu=WU, gx_via=GX_VIA):
    """Build the SPMD single-core program (same on all 8 cores)."""
    DT, _ = _dts(dt_name)
    F32 = mybir.dt.float32
    nch = tw // c
    npc = nch + 1

    nc = bacc.Bacc("TRN2", target_bir_lowering=False, debug=False,
                   num_devices=NCORES)

    # ---- DRAM I/O (per-core data; program identical across cores) ----
    xts = [nc.dram_tensor(f"xt{w}", [D + 1, (tw + c) * B], DT,
                          kind="ExternalInput").ap() for w in range(KW)]
    wa_d = nc.dram_tensor("wa", [D + 1, 3 * H1], DT, kind="ExternalInput").ap()
    whh1_d = nc.dram_tensor("whh1", [H1, 3 * H1], DT, kind="ExternalInput").ap()
    wb_d = nc.dram_tensor("wb", [H1, 3 * H2], DT, kind="ExternalInput").ap()
    b2col_d = nc.dram_tensor("b2col", [128, 3], F32, kind="ExternalInput").ap()
    whh2_d = nc.dram_tensor("whh2", [H2, 3 * H2], DT, kind="ExternalInput").ap()
    bhn3_d = nc.dram_tensor("bhn3", [3, 128], DT, kind="ExternalInput").ap()
    ones3_d = nc.dram_tensor("ones3", [3, SW], DT, kind="ExternalInput").ap()
    idn_d = nc.dram_tensor("idn", [128, 128], DT, kind="ExternalInput").ap()
    maskb_d = nc.dram_tensor("maskb", [128, npc * 2 * B], F32,
                             kind="ExternalInput").ap()
    pmax_d = nc.dram_tensor("pmax", [128, 2 * B], F32, kind="ExternalOutput").ap()

    with tile.TileContext(nc) as tc, ExitStack() as ctx:
        singles = ctx.enter_context(tc.tile_pool(name="singles", bufs=1))
        work = ctx.enter_context(tc.tile_pool(name="work", bufs=2))
        xcp = ctx.enter_context(tc.tile_pool(name="xc", bufs=3))
        gxp = ctx.enter_context(tc.tile_pool(name="gx", bufs=2))
        hbp = ctx.enter_context(tc.tile_pool(name="hb", bufs=2))
        gps = ctx.enter_context(tc.tile_pool(name="gps", bufs=2, space="PSUM"))
        php = ctx.enter_context(tc.tile_pool(name="php", bufs=2, space="PSUM"))

        # ---- load constants into SBUF ----
        wa_sb = singles.tile([D + 1, 3 * H1], DT)
        nc.sync.dma_start(wa_sb[:], wa_d[:])
        whh1_sb = singles.tile([128, 2 * 3 * H1], DT)  # [128, 1536]: k0|k1
        nc.sync.dma_start(whh1_sb[:, 0:768], whh1_d[0:128, :])
        nc.sync.dma_start(whh1_sb[:, 768:1536], whh1_d[128:256, :])
        wb_sb = singles.tile([128, 2 * 3 * H2], DT)    # [128, 768]: k0|k1
        nc.sync.dma_start(wb_sb[:, 0:384], wb_d[0:128, :])
        nc.sync.dma_start(wb_sb[:, 384:768], wb_d[128:256, :])
        b2col_sb = singles.tile([128, 3], F32)
        nc.sync.dma_start(b2col_sb[:], b2col_d[:])
        whh2_sb = singles.tile([H2, 3 * H2], DT)
        nc.sync.dma_start(whh2_sb[:], whh2_d[:])
        bhn3_sb = singles.tile([3, 128], DT)
        nc.sync.dma_start(bhn3_sb[:], bhn3_d[:])
        ones3_sb = singles.tile([3, SW], DT)
        nc.sync.dma_start(ones3_sb[:], ones3_d[:])
        idn_sb = singles.tile([128, 128], DT)
        nc.sync.dma_start(idn_sb[:], idn_d[:])
        maskb_sb = singles.tile([128, npc * 2 * B], F32)
        nc.sync.dma_start(maskb_sb[:], maskb_d[:])
        pmax_sb = singles.tile([128, 2 * B], F32)
        nc.vector.memset(pmax_sb[:], -2.0 * BIG)

        def phase_a(gxc, xcs_pair):
            """gx1 for a chunk: per window w, 6 matmuls (m-tiles of W_ih1)
            + copies into gxc L1 sections. Copies on scalar/gpsimd."""
            for w in range(KW):
                for m in range(6):
                    g, hs = m // 2, m % 2
                    ps = php.tile([128, c * B], F32, tag="ph", name="ph")
                    nc.tensor.matmul(ps[:], wa_sb[:, m * 128:(m + 1) * 128],
                                     xcs_pair[w][:], start=True, stop=True)
                    dst = gxc[:, :, g, hs * 128 + w * B:hs * 128 + (w + 1) * B]
                    src = ps[:].rearrange("p (t b) -> p t b", t=c)
                    if m % 2 == 0:
                        nc.scalar.activation(dst, src,
                                             mybir.ActivationFunctionType.Copy)
                    else:
                        nc.vector.tensor_copy(dst, src)

        def phase_b(gxc, hb):
            """gx2 for L2 from h1 written this chunk; writes gxc(next) L2
            sections. Per (w, m): 2 matmuls + 1 bias-folding copy."""
            for w in range(KW):
                h1k0 = hb[:, 1:c + 1, 0 + w, :]
                h1k1 = hb[:, 1:c + 1, 2 + w, :]
                for m in range(3):
                    ps = php.tile([128, c * B], F32, tag="ph", name="ph")
                    nc.tensor.matmul(ps[:], wb_sb[:, m * 128:(m + 1) * 128],
                                     h1k0, start=True, stop=False)
                    nc.tensor.matmul(ps[:], wb_sb[:, 384 + m * 128:384 + (m + 1) * 128],
                                     h1k1, start=False, stop=True)
                    dst = gxc[:, :, m, 256 + w * B:256 + (w + 1) * B]
                    nc.vector.tensor_scalar(
                        dst, ps[:].rearrange("p (t b) -> p t b", t=c),
                        b2col_sb[:, m:m + 1], None, op0=mybir.AluOpType.add)

        # ---- prologue ----
        xcs = [[None] * (nch + 1) for _ in range(KW)]
        for w in range(KW):
            for k in (0, 1):
                xcs[w][k] = xcp.tile([D + 1, c * B], DT, tag=f"xc{w}", name=f"xc{w}")
                nc.sync.dma_start(xcs[w][k][:], xts[w][:, k * c * B:(k + 1) * c * B])
        gxc_cur = gxp.tile([128, c, 3, SW], DT, tag="gx", name="gx")
        nc.vector.memset(gxc_cur[:], 0.0)
        # junk chunk 0: clamp L2 z-gate input so z=1 and h2 stays exactly 0
        nc.vector.memset(gxc_cur[:, :, 1, 256:384], BIG)
        phase_a(gxc_cur, [xcs[0][0], xcs[1][0]])
        hb_cur = hbp.tile([128, c + 1, 6, B], DT, tag="hb", name="hb")
        nc.vector.memset(hb_cur[:, 0, :, :], 0.0)

        # ---- main loop over processing chunks ----
        for k in range(npc):
            gxc_next = None
            # prefetch x for chunk k+2; phase-A for chunk k+1
            for w in range(KW):
                if k + 2 <= nch:
                    xcs[w][k + 2] = xcp.tile([D + 1, c * B], DT, tag=f"xc{w}", name=f"xc{w}")
                    nc.sync.dma_start(xcs[w][k + 2][:],
                                      xts[w][:, (k + 2) * c * B:(k + 3) * c * B])
            if k + 1 <= nch:
                gxc_next = gxp.tile([128, c, 3, SW], DT, tag="gx", name="gx")
                phase_a(gxc_next, [xcs[0][k + 1], xcs[1][k + 1]])

            # steps
            for s in range(c):
                Pr = gps.tile([128, SW], F32, tag="Pr", name="Pr")
                Pz = gps.tile([128, SW], F32, tag="Pz", name="Pz")
                Pn = gps.tile([128, SW], F32, tag="Pn", name="Pn")
                srz = work.tile([128, 2 * SW], DT, tag="srz", name="srz")
                tn = work.tile([128, SW], DT, tag="tn", name="tn")
                tn2 = work.tile([128, SW], DT, tag="tn2", name="tn2")
                ntl = work.tile([128, SW], DT, tag="ntl", name="ntl")
                wzh = work.tile([128, SW], DT, tag="wzh", name="wzh")
                u = work.tile([128, SW], DT, tag="u", name="u")

                h_s = hb_cur[:, s, :, :]        # [128, 6, B]
                hk0 = hb_cur[:, s, 0:2, :]      # L1 k-half 0, both windows
                hk1 = hb_cur[:, s, 2:4, :]      # L1 k-half 1
                h2s = hb_cur[:, s, 4:6, :]      # L2 state

                def l1_bank(P, m0, first_start):
                    """4 L1 matmuls (2 m-tiles x 2 k-halves) into bank P."""
                    for mi, m in enumerate((m0, m0 + 1)):
                        nc.tensor.matmul(P[:, mi * 128:(mi + 1) * 128],
                                         whh1_sb[:, m * 128:(m + 1) * 128],
                                         hk0, start=first_start, stop=False)
                        nc.tensor.matmul(P[:, mi * 128:(mi + 1) * 128],
                                         whh1_sb[:, 768 + m * 128:768 + (m + 1) * 128],
                                         hk1, start=False, stop=False)

                # ---- r bank ----
                if gx_via == "tensor":
                    nc.tensor.matmul(Pr[:], idn_sb[:], gxc_cur[:, s, 0, :],
                                     start=True, stop=False)
                    l1_bank(Pr, 0, False)
                else:
                    l1_bank(Pr, 0, True)
                nc.tensor.matmul(Pr[:, 256:384], whh2_sb[:, 0:128], h2s,
                                 start=(gx_via != "tensor"), stop=True)
                if gx_via == "tensor":
                    nc.scalar.activation(srz[:, 0:SW], Pr[:],
                                         mybir.ActivationFunctionType.Sigmoid)
                else:
                    nc.vector.tensor_add(srz[:, 0:SW], Pr[:], gxc_cur[:, s, 0, :])
                    nc.scalar.activation(srz[:, 0:SW], srz[:, 0:SW],
                                         mybir.ActivationFunctionType.Sigmoid)
                # ---- n bank (bias via K=3 selector matmul) ----
                nc.tensor.matmul(Pn[:], bhn3_sb[:], ones3_sb[:],
                                 start=True, stop=False)
                l1_bank(Pn, 4, False)
                nc.tensor.matmul(Pn[:, 256:384], whh2_sb[:, 256:384], h2s,
                                 start=False, stop=True)
                # ---- z bank ----
                if gx_via == "tensor":
                    nc.tensor.matmul(Pz[:], idn_sb[:], gxc_cur[:, s, 1, :],
                                     start=True, stop=False)
                    l1_bank(Pz, 2, False)
                else:
                    l1_bank(Pz, 2, True)
                nc.tensor.matmul(Pz[:, 256:384], whh2_sb[:, 128:256], h2s,
                                 start=(gx_via != "tensor"), stop=True)
                if gx_via == "tensor":
                    nc.scalar.activation(srz[:, SW:2 * SW], Pz[:],
                                         mybir.ActivationFunctionType.Sigmoid)
                else:
                    nc.vector.tensor_add(srz[:, SW:2 * SW], Pz[:],
                                         gxc_cur[:, s, 1, :])
                    nc.scalar.activation(srz[:, SW:2 * SW], srz[:, SW:2 * SW],
                                         mybir.ActivationFunctionType.Sigmoid)
                # ---- n path + state update ----
                nc.vector.tensor_mul(tn[:], srz[:, 0:SW], Pn[:])
                nc.vector.tensor_add(tn2[:], tn[:], gxc_cur[:, s, 2, :])
                nc.scalar.activation(ntl[:], tn2[:],
                                     mybir.ActivationFunctionType.Tanh)
                nc.gpsimd.tensor_mul(wzh[:], srz[:, SW:2 * SW], h_s)
                nc.vector.scalar_tensor_tensor(
                    u[:], srz[:, SW:2 * SW], 1.0, ntl[:],
                    op0=mybir.AluOpType.subtract, op1=mybir.AluOpType.mult)
                nc.vector.tensor_sub(hb_cur[:, s + 1, :, :], wzh[:], u[:])

            # phase-B, carry, pooling
            if k <= nch - 1:
                phase_b(gxc_next, hb_cur)
            if k + 1 <= npc - 1:
                hnew = hbp.tile([128, c + 1, 6, B], DT, tag="hb", name="hb")
                nc.gpsimd.tensor_copy(hnew[:, 0, :, :], hb_cur[:, c, :, :])
            else:
                hnew = None
            # pooling over this chunk's h2 (L2 steps (k-1)*C .. k*C-1)
            cmax = work.tile([128, 2 * B], F32, tag="cmax", name="cmax")
            nc.vector.tensor_reduce(
                cmax[:], hb_cur[:, 1:c + 1, 4:6, :].rearrange("p t s b -> p s b t"),
                axis=mybir.AxisListType.X, op=mybir.AluOpType.max)
            nc.vector.tensor_add(cmax[:], cmax[:],
                                 maskb_sb[:, k * 2 * B:(k + 1) * 2 * B])
            nc.vector.tensor_max(pmax_sb[:], pmax_sb[:], cmax[:])
            if hnew is not None:
                hb_cur = hnew
            gxc_cur = gxc_next

        # ---- epilogue ----
        nc.sync.dma_start(pmax_d[:], pmax_sb[:])

    nc.compile()
    return nc


def prep_core_inputs(inputs, dt_name=DT_NAME, tw=TW, c=C, wu=WU,
                     winlen=WINLEN):
    """Host-side data prep: per-core input dicts (layout/slice/cast only)."""
    _, NPD = _dts(dt_name)
    nch = tw // c
    npc = nch + 1
    x = np.asarray(inputs['x'], np.float32)
    W_ih1 = np.asarray(inputs['W_ih1'], np.float32)
    W_hh1 = np.asarray(inputs['W_hh1'], np.float32)
    b_ih1 = np.asarray(inputs['b_ih1'], np.float32)
    b_hh1 = np.asarray(inputs['b_hh1'], np.float32)
    W_ih2 = np.asarray(inputs['W_ih2'], np.float32)
    W_hh2 = np.asarray(inputs['W_hh2'], np.float32)
    b_ih2 = np.asarray(inputs['b_ih2'], np.float32)
    b_hh2 = np.asarray(inputs['b_hh2'], np.float32)
    g1, be1 = np.asarray(inputs['bn1_gamma'], np.float32), np.asarray(inputs['bn1_beta'], np.float32)
    m1, v1 = np.asarray(inputs['bn1_mean'], np.float32), np.asarray(inputs['bn1_var'], np.float32)

    s1 = g1 / np.sqrt(v1 + EPS)
    W2p = W_ih2 * s1[None, :]                      # [384, 256] scaled
    b2extra = W_ih2 @ (be1 - m1 * s1)              # [384]
    b2row = (b2extra + b_ih2 +
             np.concatenate([b_hh2[0:H2], b_hh2[H2:2 * H2], np.zeros(H2, np.float32)]))
    b2col = np.stack([b2row[0:128], b2row[128:256], b2row[256:384]], axis=1)  # [128,3]
    wa = np.vstack([
        W_ih1.T,
        (b_ih1 + np.concatenate([b_hh1[0:H1], b_hh1[H1:2 * H1],
                                 np.zeros(H1, np.float32)]))[None, :],
    ])  # [65, 768]
    # n-gate hidden bias: K=3 selector matmul operands
    bhn3 = np.stack([b_hh1[2 * H1:2 * H1 + 128],
                     b_hh1[2 * H1 + 128:3 * H1],
                     b_hh2[2 * H2:3 * H2]], axis=0)          # [3, 128]
    ones3 = np.zeros((3, SW), np.float32)
    for sct in range(3):
        ones3[sct, sct * 128:(sct + 1) * 128] = 1.0

    base = dict(
        wa=wa.astype(NPD),
        whh1=W_hh1.T.astype(NPD).copy(),
        wb=W2p.T.astype(NPD).copy(),
        b2col=b2col.astype(np.float32).copy(),
        whh2=W_hh2.T.astype(NPD).copy(),
        bhn3=bhn3.astype(NPD).copy(),
        ones3=ones3.astype(NPD),
        idn=np.eye(128, dtype=np.float32).astype(NPD),
    )

    in_maps = []
    for core in range(NCORES):
        m = dict(base)
        maskb = np.zeros((128, npc * 2 * B), np.float32)
        for w in range(KW):
            widx = core * KW + w
            t0 = 0 if widx == 0 else widx * winlen - wu
            xw = x[:, t0:t0 + tw, :]                       # [64, TW, 64]
            xt = np.transpose(xw, (2, 1, 0)).reshape(D, tw * B)
            xt = np.concatenate([xt, np.zeros((D, c * B), np.float32)], axis=1)
            xt = np.vstack([xt, np.ones((1, (tw + c) * B), np.float32)])
            m[f"xt{w}"] = np.ascontiguousarray(xt).astype(NPD)
            maskb[:, 0 * 2 * B + w * B:0 * 2 * B + (w + 1) * B] = -BIG  # junk chunk
            if widx > 0:
                for kk in range(1, 1 + wu // c):
                    maskb[:, kk * 2 * B + w * B:kk * 2 * B + (w + 1) * B] = -BIG
        m["maskb"] = maskb
        in_maps.append(m)
    return in_maps


def finalize(pmax_list, inputs):
    """Host: combine per-core pooled maxima, apply BN2 + tanh + FC."""
    allp = np.stack(pmax_list)                             # [ncores, 128, 2B]
    pmax = np.max(allp.reshape(NCORES, 128, KW, B), axis=(0, 2))   # [128, B]
    g2 = np.asarray(inputs['bn2_gamma'], np.float32)
    be2 = np.asarray(inputs['bn2_beta'], np.float32)
    m2 = np.asarray(inputs['bn2_mean'], np.float32)
    v2 = np.asarray(inputs['bn2_var'], np.float32)
    fc_w = np.asarray(inputs['fc_w'], np.float32)
    fc_b = np.asarray(inputs['fc_b'], np.float32)
    s2 = g2 / np.sqrt(v2 + EPS)
    th = np.tanh(pmax * s2[:, None] + (be2 - m2 * s2)[:, None])   # [128, 64]
    return (th.T @ fc_w.T + fc_b).astype(np.float32)               # [64, 10]


_NC_CACHE = {}


def _get_nc(dt_name=DT_NAME):
    if dt_name not in _NC_CACHE:
        _NC_CACHE[dt_name] = build_bass(dt_name)
    return _NC_CACHE[dt_name]


def kernel(**inputs):
    nc = _get_nc()
    in_maps = prep_core_inputs(inputs)
    res = run_bass_kernel_spmd(nc, in_maps, list(range(NCORES)))
    pmax_list = [res.results[i]["pmax"] for i in range(NCORES)]
    return finalize(pmax_list, inputs)
